# revision 14
# baseline (speedup 1.0000x reference)
"""DetectHead (three 1x1-conv heads fused) on 8 Trainium2 NeuronCores.

Math: out[b,h,w,:] = concat(cls, box, dir) = W_all @ x[b,:,h,w] + bias_all
with W_all = concat(cls_w, box_w, dir_w) in R^{72x1024}.

Sharding: 8 shards = (batch, H-half). Each core processes a contiguous
(1024, 100*176=17600) slice of x and produces (17600, 72) of the
channels-last output.

The kernel is HBM/PE-bound, so precision is the main lever. Two modes:

bf16 : x and W cast to bf16 on host (exact rel-err vs the fp64 oracle on
       the real inputs: 1.0e-3, 20x inside the 2e-2 gate); fp16 output.
       36 MB in + 2.5 MB out per core.
fp8  : x*32 and per-row-scaled W cast to fp8 e4m3 (TRN variant, max 240;
       ml_dtypes.float8_e4m3 matches bit-for-bit). Exact rel-err on the
       real inputs: 1.64e-2 fp32-out (+~2e-4 fp16-out), inside the 2e-2
       gate. DoubleRow matmuls contract 256 channels/pass by pairing
       channel chunks (k, k+4): weights [128,2,72] APs with pair stride
       288 B, moving [128,2,512] APs with pair stride 4*GROUP B.
       Per-row dequant (1/(32*sw_o), exact powers of 2) and bias are
       fused into one DVE tensor_scalar (mult+add) before the PE
       transposes. 18 MB in + 2.5 MB out per core.

Per 512-pixel tile: matmuls -> PSUM [72,512] fp32, DVE dequant+bias ->
SBUF, 4 PE transposes ([72,128] -> [128,72]), DVE copy -> fp16, DMA out
on the ACT HWDGE ring ([512,72] pixel-major contiguous).
"""

import numpy as np
import ml_dtypes
from contextlib import ExitStack

import concourse.bass as bass
import concourse.tile as tile
from concourse import bacc, mybir
from concourse.bass_utils import run_bass_kernel_spmd

B, C, H, W = 4, 1024, 200, 176
HH = H // 2            # 100 rows of H per shard
PIX = HH * W           # 17600 pixels per shard
NCORES = 8
KCH = C // 128         # 8 channel chunks
O = 72                 # 18 cls + 42 box + 12 dir output channels
TILE_N = 512

F32 = mybir.dt.float32
F16 = mybir.dt.float16
BF16 = mybir.dt.bfloat16
FP8 = mybir.dt.float8e4

MODE = "fp8"           # "bf16" or "fp8"
GROUP_DEF = 4096 if MODE == "fp8" else 2048
XSCALE = 32.0

_compiled = {}


def _build_program(mode=MODE, repeat=1, group=GROUP_DEF, xbufs=4):
    xdt = FP8 if mode == "fp8" else BF16
    nc = bacc.Bacc(
        "TRN2", target_bir_lowering=False, debug=False, num_devices=NCORES
    )
    xs = nc.dram_tensor("xs", [C, PIX], xdt, kind="ExternalInput").ap()
    wt = nc.dram_tensor("wt", [128, KCH * O], xdt, kind="ExternalInput").ap()
    dqb = nc.dram_tensor("dqb", [O, 2], F32, kind="ExternalInput").ap()
    biasbc = nc.dram_tensor("biasbc", [128, 4 * O], F32, kind="ExternalInput").ap()
    ident = nc.dram_tensor("ident", [O, O], F16, kind="ExternalInput").ap()
    out = nc.dram_tensor("out", [PIX, O], F16, kind="ExternalOutput").ap()

    # [c, pix] viewed as [p, k, pix] with c = k*128 + p
    xs_v = xs.rearrange("(k p) n -> p k n", k=KCH)

    with tile.TileContext(nc) as tc, ExitStack() as ctx:
        cpool = ctx.enter_context(tc.tile_pool(name="consts", bufs=1))
        xpool = ctx.enter_context(tc.tile_pool(name="xin", bufs=xbufs))
        spool = ctx.enter_context(tc.tile_pool(name="stage", bufs=3))
        opool = ctx.enter_context(tc.tile_pool(name="outsb", bufs=3))
        mpool = ctx.enter_context(tc.tile_pool(name="pmm", bufs=2, space="PSUM"))
        tpool = ctx.enter_context(tc.tile_pool(name="ptr", bufs=2, space="PSUM"))

        w_sb = cpool.tile([128, KCH * O], xdt)
        nc.sync.dma_start(out=w_sb[:, :], in_=wt[:, :])
        dq_sb = cpool.tile([O, 2], F32)
        nc.sync.dma_start(out=dq_sb[:, :], in_=dqb[:, :])
        bias_sb = cpool.tile([128, 4 * O], F32)
        nc.sync.dma_start(out=bias_sb[:, :], in_=biasbc[:, :])
        id_sb = cpool.tile([O, O], F16)
        nc.sync.dma_start(out=id_sb[:, :], in_=ident[:, :])

        w_pair = w_sb[:, :].rearrange("p (i r) -> p i r", i=2)  # [128,2,KCH//2*O]

        def do_mm_tile(xbuf, off, pix0, n):
            # one matmul pipeline over n<=512 pixels at offset `off` in xbuf
            njs = [128] * (n // 128)
            if n % 128:
                njs.append(n % 128)
            nj = len(njs)

            pmm = mpool.tile([O, n], F32, tag="pmm")
            if mode == "fp8":
                x_pair = xbuf[:, :, off : off + n].rearrange(
                    "p (i j) n -> p i j n", i=2
                )  # [128, 2, KCH//2, n]
                for j in range(KCH // 2):
                    nc.tensor.matmul(
                        pmm[:, :],
                        w_pair[:, :, j * O : (j + 1) * O],
                        x_pair[:, :, j, :],
                        start=(j == 0),
                        stop=(j == KCH // 2 - 1),
                        perf_mode=mybir.MatmulPerfMode.DoubleRow,
                    )
            else:
                for k in range(KCH):
                    nc.tensor.matmul(
                        pmm[:, :],
                        w_sb[:, k * O : (k + 1) * O],
                        xbuf[:, k, off : off + n],
                        start=(k == 0),
                        stop=(k == KCH - 1),
                    )

            # dequant (fp8) / unit-scale (bf16) + bias, fused on DVE.
            # f16 staging padded to 80 partitions / 128-multiple cols so the
            # DMA xbar transpose engine (16|p_dim, 128|free_dim) can do the
            # [72,128]->[128,72] transposes instead of the PE.
            nr = (n + 127) // 128 * 128
            s1 = spool.tile([80, nr], F16, tag="s1")
            nc.vector.tensor_scalar(
                out=s1[:O, :n],
                in0=pmm[:, :],
                scalar1=dq_sb[:, 0:1],
                scalar2=dq_sb[:, 1:2],
                op0=mybir.AluOpType.mult,
                op1=mybir.AluOpType.add,
            )

            ot = opool.tile([128, nj * 80], F16, tag="ot")
            for j in range(nj):
                nc.scalar.dma_start(
                    out=ot[:, j * 80 : (j + 1) * 80],
                    in_=s1[:, j * 128 : (j + 1) * 128],
                    transpose=True,
                )
            if n % 128 == 0:
                nc.scalar.dma_start(
                    out=out[pix0 : pix0 + n, :].rearrange("(j p) o -> p j o", p=128),
                    in_=ot[:, :].rearrange("p (j o) -> p j o", j=nj)[:, :, :O],
                )
            else:
                for j, pj in enumerate(njs):
                    nc.scalar.dma_start(
                        out=out[pix0 + j * 128 : pix0 + j * 128 + pj, :],
                        in_=ot[:pj, j * 80 : j * 80 + O],
                    )

        def do_group(pix0, n):
            # one input DMA covering n pixels (up to GROUP), then MM tiles of 512
            xbuf = xpool.tile([128, KCH, n], xdt, tag="xbuf")
            nc.sync.dma_start(out=xbuf[:, :, :], in_=xs_v[:, :, pix0 : pix0 + n])
            off = 0
            while off < n:
                m = min(TILE_N, n - off)
                do_mm_tile(xbuf, off, pix0 + off, m)
                off += m

        for _rep in range(repeat):
            g0 = 0
            while g0 < PIX:
                gn = min(group, PIX - g0)
                do_group(g0, gn)
                g0 += gn

    nc.compile()
    return nc


def _get_program(repeat=1, mode=MODE, group=GROUP_DEF, xbufs=4):
    key = (mode, repeat, group, xbufs)
    if key not in _compiled:
        _compiled[key] = _build_program(mode, repeat, group, xbufs)
    return _compiled[key]


def _make_in_maps(x, cls_w, cls_b, box_w, box_b, dir_w, dir_b, mode=MODE):
    w_all = np.concatenate(
        [np.asarray(cls_w), np.asarray(box_w), np.asarray(dir_w)], axis=0
    ).astype(np.float32)  # (72, 1024)
    bias_all = np.concatenate(
        [np.asarray(cls_b), np.asarray(box_b), np.asarray(dir_b)]
    ).astype(np.float32)  # (72,)

    x = np.asarray(x)
    if mode == "fp8":
        rowmax = np.abs(w_all).max(axis=1)
        sw = (2.0 ** np.floor(np.log2(224.0 / rowmax))).astype(np.float32)
        wq = (w_all * sw[:, None]).astype(ml_dtypes.float8_e4m3)
        dq = (1.0 / (XSCALE * sw)).astype(np.float32)
        wt = np.ascontiguousarray(
            wq.T.reshape(KCH, 128, O).transpose(1, 0, 2).reshape(128, KCH * O)
        )
        xc = (x * XSCALE).astype(ml_dtypes.float8_e4m3)
    else:
        dq = np.ones(O, dtype=np.float32)
        wt = np.ascontiguousarray(
            w_all.T.reshape(KCH, 128, O).transpose(1, 0, 2).reshape(128, KCH * O)
        ).astype(ml_dtypes.bfloat16)
        xc = x.astype(ml_dtypes.bfloat16)

    dqb = np.ascontiguousarray(np.stack([dq, bias_all], axis=1))  # (72, 2)
    biasbc = np.ascontiguousarray(np.tile(bias_all, (128, 4)))
    ident = np.eye(O, dtype=np.float16)

    in_maps = []
    for i in range(NCORES):
        b, half = divmod(i, 2)
        xsh = np.ascontiguousarray(
            xc[b, :, half * HH : (half + 1) * HH, :]
        ).reshape(C, PIX)
        in_maps.append(
            {"xs": xsh, "wt": wt, "dqb": dqb, "biasbc": biasbc, "ident": ident}
        )
    return in_maps


def _gather(results):
    out = np.empty((B, H, W, O), dtype=np.float32)
    for i in range(NCORES):
        b, half = divmod(i, 2)
        out[b, half * HH : (half + 1) * HH] = (
            results[i]["out"].astype(np.float32).reshape(HH, W, O)
        )
    return out


def kernel(x, cls_w, cls_b, box_w, box_b, dir_w, dir_b):
    nc = _get_program()
    in_maps = _make_in_maps(x, cls_w, cls_b, box_w, box_b, dir_w, dir_b)
    res = run_bass_kernel_spmd(nc, in_maps, list(range(NCORES)))
    return _gather(res.results)
